# revision 9
# baseline (speedup 1.0000x reference)
"""Gated TCN layer (fully conditioned) as a Bass/Tile kernel on 8 NeuronCores.

Reference computation (per sample b):
    kern = (c @ adapter_w + adapter_b).reshape(2*CH, CH, K)
    y    = dilated causal conv of x with per-sample kern (K=3, dil=4)
    y   += (c @ bias_w + bias_b)[:, None]
    z    = tanh(y[:CH]) * sigmoid(y[CH:])
    out  = resi_w @ z + resi_b + x
Returns (out, z).

Sharding: data-parallel over batch, 2 samples per core. The two samples are
packed on the 128 SBUF partitions (rows 64b+ch) so every activation / vector
op runs full-width. The per-sample conv kernels are laid out block-diagonally
per (gate, tap) so one matmul computes one gate half for both samples at once.
All matmul operands are bf16 (full PE rate + FWL); accumulation is fp32 PSUM.

Schedule: two-wave input DMA (tanh-gate adapter chunks + first x chunk first),
then a tanh-conv pass that overlaps the remaining loads, then a sigmoid-conv +
gate + residual pass. All large HBM transfers are chunk-major contiguous.
"""

import numpy as np

from concourse import bacc, mybir, tile
from concourse.bass_utils import run_bass_kernel_spmd

K = 3
DIL = 4
CH = 64
COND = 128
B, T = 16, 16384
NCORES = 8
BL = B // NCORES          # samples per core
PAD = (K - 1) * DIL       # causal left pad = 8
NT = 512                  # matmul free-dim (one PSUM bank of fp32)
UW = 1024                 # processing unit width (2 PSUM banks)
NJ = T // UW
F = K * CH * 2 * CH       # 24576 adapter columns
QCH = CH * CH             # 4096 adapter columns per (gate, tap) block
NXC = 4                   # x load chunks
XC = T // NXC
QLIST = [0, 2, 4, 1, 3, 5]  # chunk order: tanh-gate (even q) blocks first

F32 = mybir.dt.float32
BF16 = mybir.dt.bfloat16
AF = mybir.ActivationFunctionType
ALU = mybir.AluOpType

# Set by test.py to capture a profile; harness path leaves these alone.
TRACE = False
LAST_RESULTS = None

_NC = None


def _build():
    nc = bacc.Bacc("TRN2", target_bir_lowering=False, debug=False)

    x_in = nc.dram_tensor("x_in", [NXC, 2 * CH, XC], BF16, kind="ExternalInput")
    cT_d = nc.dram_tensor("cT", [COND, BL], BF16, kind="ExternalInput")
    aw_d = nc.dram_tensor("aw_r", [6, COND, QCH], BF16, kind="ExternalInput")
    ab_d = nc.dram_tensor("ab_p", [2 * CH, 6 * 128], BF16, kind="ExternalInput")
    bw_d = nc.dram_tensor("bw", [COND, 2 * CH], BF16, kind="ExternalInput")
    bbt_d = nc.dram_tensor("bbt", [2 * CH, 1], F32, kind="ExternalInput")
    bbs_d = nc.dram_tensor("bbs", [2 * CH, 1], F32, kind="ExternalInput")
    rwT_d = nc.dram_tensor("rwT", [2 * CH, 2 * CH], BF16, kind="ExternalInput")
    rb_d = nc.dram_tensor("rb", [2 * CH, 1], F32, kind="ExternalInput")
    out_d = nc.dram_tensor("out_d", [NJ, 2 * CH, UW], BF16, kind="ExternalOutput")
    z_d = nc.dram_tensor("z_d", [NJ, 2 * CH, UW], BF16, kind="ExternalOutput")

    with tile.TileContext(nc) as tc:
        with (
            tc.tile_pool(name="const", bufs=1) as constp,
            tc.tile_pool(name="xpool", bufs=1) as xpool,
            tc.tile_pool(name="kern", bufs=1) as kernp,
        ):
            cT_sb = constp.tile([COND, BL], BF16)
            nc.sync.dma_start(cT_sb[:, :], cT_d[:, :])
            bw_sb = constp.tile([COND, 2 * CH], BF16)
            nc.sync.dma_start(bw_sb[:, :], bw_d[:, :])

            # wave 1: tanh-gate adapter chunks + first x chunk
            awts = []
            for cidx in range(6):
                awt = constp.tile([COND, QCH], BF16, name=f"awt{cidx}", tag=f"aw{cidx}")
                awts.append(awt)
            for cidx in range(3):
                nc.sync.dma_start(awts[cidx][:, :], aw_d[cidx])

            xz = xpool.tile([2 * CH, PAD + T], BF16)
            nc.vector.memset(xz[:, 0:PAD].bitcast(F32), 0.0)
            nc.sync.dma_start(xz[:, PAD : PAD + XC], x_in[0])

            ab_sb = constp.tile([2 * CH, 6 * 128], BF16)
            nc.sync.dma_start(ab_sb[:, :], ab_d[:, :])
            rwT_sb = constp.tile([2 * CH, 2 * CH], BF16)
            nc.sync.dma_start(rwT_sb[:, :], rwT_d[:, :])
            rb_sb = constp.tile([2 * CH, 1], F32)
            nc.sync.dma_start(rb_sb[:, :], rb_d[:, :])
            bbt_sb = constp.tile([2 * CH, 1], F32)
            nc.sync.dma_start(bbt_sb[:, :], bbt_d[:, :])
            bbs_sb = constp.tile([2 * CH, 1], F32)
            nc.sync.dma_start(bbs_sb[:, :], bbs_d[:, :])

            # Block-diagonal per-(gate,tap) kernel tiles: block q=2k+g holds
            # lhsT[64b+i, 64b+o'] = kern[b, g*64+o', i, k].
            kern_raw = kernp.tile([2 * CH, 6 * 128], BF16, name="kern_raw")
            nc.vector.memset(kern_raw[:, :].bitcast(F32), 0.0)
            kern = kernp.tile([2 * CH, 6 * 128], BF16, name="kern")
            bias_t = kernp.tile([2 * CH, 1], F32)
            bias_s = kernp.tile([2 * CH, 1], F32)
            # tanh-gate activations for all tiles (pass 1 output)
            ta_all = xpool.tile([2 * CH, T], BF16, name="ta_all")

            # ---------------- phase A: conditioned bias ---------------------
            with (
                tc.tile_pool(name="bps", bufs=1, space="PSUM") as bpsp,
                tc.tile_pool(name="bstg", bufs=1) as bstgp,
            ):
                pb = bpsp.tile([2 * CH, BL], F32)
                nc.tensor.matmul(pb[:, :], bw_sb[:, :], cT_sb[:, :], start=True, stop=True)
                pbs = bstgp.tile([2 * CH, BL], F32)
                nc.vector.tensor_copy(pbs[:, :], pb[:, :])
                # pair layout: rows 64b+o' = bias for sample b, out-chan o'
                nc.sync.dma_start(bias_t[0:CH, :], pbs[0:CH, 0:1])
                nc.sync.dma_start(bias_t[CH : 2 * CH, :], pbs[0:CH, 1:2])
                nc.sync.dma_start(bias_s[0:CH, :], pbs[CH : 2 * CH, 0:1])
                nc.sync.dma_start(bias_s[CH : 2 * CH, :], pbs[CH : 2 * CH, 1:2])
                nc.vector.tensor_add(bias_t[:, :], bias_t[:, :], bbt_sb[:, :])
                nc.vector.tensor_add(bias_s[:, :], bias_s[:, :], bbs_sb[:, :])

            # ------------- phase A adapter chunks + pass-1 tanh conv --------
            with (
                tc.tile_pool(name="apsum", bufs=1, space="PSUM") as apsum,
                tc.tile_pool(name="stg", bufs=3) as stgp,
                tc.tile_pool(name="tpsum", bufs=2, space="PSUM") as tpsum,
            ):
                def adapter_chunk(cidx):
                    q = QLIST[cidx]
                    awt = awts[cidx]
                    for h2 in range(2):
                        ps = apsum.tile([BL, 2048], F32, tag="aps")
                        for v in range(4):
                            nc.tensor.matmul(
                                ps[:, 512 * v : 512 * (v + 1)],
                                cT_sb[:, :],
                                awt[:, 2048 * h2 + 512 * v : 2048 * h2 + 512 * (v + 1)],
                                start=True,
                                stop=True,
                            )
                        # drain PSUM with scalar and vector in parallel halves
                        stg = stgp.tile([BL, 2048], BF16, tag="stg")
                        nc.scalar.activation(stg[:, 0:1024], ps[:, 0:1024], AF.Copy)
                        nc.vector.tensor_copy(stg[:, 1024:2048], ps[:, 1024:2048])
                        for b in range(BL):
                            nc.sync.dma_start(
                                kern_raw[
                                    CH * b + 32 * h2 : CH * b + 32 * h2 + 32,
                                    128 * q + CH * b : 128 * q + CH * b + CH,
                                ],
                                stg[b : b + 1, :],
                            )
                    nc.vector.tensor_add(
                        kern[:, 128 * q : 128 * (q + 1)],
                        kern_raw[:, 128 * q : 128 * (q + 1)],
                        ab_sb[:, 128 * q : 128 * (q + 1)],
                    )

                for cidx in range(3):
                    adapter_chunk(cidx)
                    # wave 2 loads fire as the tanh-gate chunks are consumed
                    nc.sync.dma_start(awts[cidx + 3][:, :], aw_d[cidx + 3])
                    nc.sync.dma_start(
                        xz[:, PAD + (cidx + 1) * XC : PAD + (cidx + 2) * XC],
                        x_in[cidx + 1],
                    )

                for j in range(NJ):
                    pt = tpsum.tile([2 * CH, UW], F32, tag="pt")
                    for k in range(K):
                        q = 2 * k
                        for h in range(UW // NT):
                            c0 = j * UW + h * NT + DIL * k
                            nc.tensor.matmul(
                                pt[:, h * NT : (h + 1) * NT],
                                kern[:, 128 * q : 128 * (q + 1)],
                                xz[:, c0 : c0 + NT],
                                start=(k == 0),
                                stop=(k == K - 1),
                            )
                    nc.scalar.activation(
                        ta_all[:, j * UW : (j + 1) * UW],
                        pt[:, :],
                        AF.Tanh,
                        bias=bias_t[:, 0:1],
                    )
                    # sig-gate adapter chunks slot into the pass-1 stream
                    if j in (3, 6, 9):
                        adapter_chunk(3 + (j - 3) // 3)

            # ---------------- pass 2: sig conv + gate + residual ------------
            with (
                tc.tile_pool(name="spsum", bufs=2, space="PSUM") as spsum,
                tc.tile_pool(name="opsum", bufs=2, space="PSUM") as opsum,
                tc.tile_pool(name="work", bufs=3) as workp,
            ):
                def emit_residual(j, zz):
                    po = opsum.tile([2 * CH, UW], F32, tag="po")
                    for h in range(UW // NT):
                        nc.tensor.matmul(
                            po[:, h * NT : (h + 1) * NT],
                            rwT_sb[:, :],
                            zz[:, h * NT : (h + 1) * NT],
                            start=True,
                            stop=True,
                        )
                    ot = workp.tile([2 * CH, UW], BF16, tag="ot")
                    nc.vector.scalar_tensor_tensor(
                        ot[:, :],
                        po[:, :],
                        rb_sb[:, 0:1],
                        xz[:, j * UW + PAD : j * UW + PAD + UW],
                        ALU.add,
                        ALU.add,
                    )
                    nc.sync.dma_start(out_d[j], ot[:, :])

                prev = None
                for j in range(NJ):
                    psg = spsum.tile([2 * CH, UW], F32, tag="ps")
                    for k in range(K):
                        q = 2 * k + 1
                        for h in range(UW // NT):
                            c0 = j * UW + h * NT + DIL * k
                            nc.tensor.matmul(
                                psg[:, h * NT : (h + 1) * NT],
                                kern[:, 128 * q : 128 * (q + 1)],
                                xz[:, c0 : c0 + NT],
                                start=(k == 0),
                                stop=(k == K - 1),
                            )
                    # residual of previous tile: its zz is ready by now, and
                    # emitting it here keeps the PE stream dense.
                    if prev is not None:
                        emit_residual(*prev)
                    ts = workp.tile([2 * CH, UW], BF16, tag="ts")
                    nc.scalar.activation(
                        ts[:, :], psg[:, :], AF.Sigmoid, bias=bias_s[:, 0:1]
                    )
                    zz = workp.tile([2 * CH, UW], BF16, tag="zz")
                    nc.vector.tensor_mul(
                        zz[:, 0:NT], ta_all[:, j * UW : j * UW + NT], ts[:, 0:NT]
                    )
                    nc.gpsimd.tensor_mul(
                        zz[:, NT:UW], ta_all[:, j * UW + NT : (j + 1) * UW], ts[:, NT:UW]
                    )
                    nc.sync.dma_start(z_d[j], zz[:, :])
                    prev = (j, zz)
                emit_residual(*prev)

    nc.compile()
    return nc


def get_nc():
    global _NC
    if _NC is None:
        _NC = _build()
    return _NC


def make_in_maps(inputs):
    import ml_dtypes

    bf = ml_dtypes.bfloat16

    x = np.asarray(inputs["x"], np.float32)
    c = np.asarray(inputs["c"], np.float32)
    aw = np.asarray(inputs["adapter_w"], np.float32)
    ab = np.asarray(inputs["adapter_b"], np.float32)
    bw = np.asarray(inputs["bias_w"], np.float32)
    bb = np.asarray(inputs["bias_b"], np.float32)
    rw = np.asarray(inputs["resi_w"], np.float32)
    rb = np.asarray(inputs["resi_b"], np.float32)

    # adapter cols [cond, (g,o',i,k)] -> chunks of (i, o') per q=2k+g in QLIST order
    aw4 = aw.reshape(COND, 2, CH, CH, K)
    chunks = []
    for cidx in range(6):
        q = QLIST[cidx]
        g, k = q % 2, q // 2
        blk = aw4[:, g, :, :, k]  # [cond, o', i]
        chunks.append(np.ascontiguousarray(blk.transpose(0, 2, 1)).reshape(COND, QCH))
    aw_r = np.ascontiguousarray(np.stack(chunks).astype(bf))  # [6, cond, QCH]

    # adapter bias in the block-diagonal pair layout (zeros off-diagonal)
    ab4 = ab.reshape(2, CH, CH, K)
    ab_p = np.zeros((2 * CH, 6 * 128), np.float32)
    for q in range(6):
        g, k = q % 2, q // 2
        blk = ab4[g, :, :, k].T  # [i, o']
        for b2 in range(BL):
            ab_p[CH * b2 : CH * (b2 + 1), 128 * q + CH * b2 : 128 * q + CH * b2 + CH] = blk
    ab_p = np.ascontiguousarray(ab_p.astype(bf))

    rwT_p = np.zeros((2 * CH, 2 * CH), np.float32)
    rwT_p[0:CH, 0:CH] = rw.T
    rwT_p[CH:, CH:] = rw.T
    rwT_p = np.ascontiguousarray(rwT_p.astype(bf))

    bbt = np.ascontiguousarray(np.tile(bb[0:CH], 2).reshape(2 * CH, 1))
    bbs = np.ascontiguousarray(np.tile(bb[CH:], 2).reshape(2 * CH, 1))
    rbp = np.ascontiguousarray(np.tile(rb, 2).reshape(2 * CH, 1))
    bw_b = np.ascontiguousarray(bw.astype(bf))

    in_maps = []
    for m in range(NCORES):
        sl = slice(BL * m, BL * (m + 1))
        # chunk-major contiguous x: [NXC, 128, XC]
        xc = np.ascontiguousarray(
            x[sl].reshape(BL * CH, NXC, XC).transpose(1, 0, 2).astype(bf)
        )
        in_maps.append(
            {
                "x_in": xc,
                "cT": np.ascontiguousarray(c[sl].T.astype(bf)),
                "aw_r": aw_r,
                "ab_p": ab_p,
                "bw": bw_b,
                "bbt": bbt,
                "bbs": bbs,
                "rwT": rwT_p,
                "rb": rbp,
            }
        )
    return in_maps


def kernel(**inputs):
    global LAST_RESULTS
    nc = get_nc()
    in_maps = make_in_maps(inputs)
    res = run_bass_kernel_spmd(nc, in_maps, list(range(NCORES)), trace=TRACE)
    LAST_RESULTS = res
    out = np.empty((B, CH, T), np.float32)
    z = np.empty((B, CH, T), np.float32)
    for m in range(NCORES):
        # [NJ, 128, UW] -> [2, CH, T]
        o = res.results[m]["out_d"].astype(np.float32)
        zt = res.results[m]["z_d"].astype(np.float32)
        out[BL * m : BL * (m + 1)] = (
            o.transpose(1, 0, 2).reshape(BL, CH, T)
        )
        z[BL * m : BL * (m + 1)] = (
            zt.transpose(1, 0, 2).reshape(BL, CH, T)
        )
    return out, z


# revision 18
# speedup vs baseline: 1.0582x; 1.0582x over previous
"""Gated TCN layer (fully conditioned) as a Bass/Tile kernel on 8 NeuronCores.

Reference computation (per sample b):
    kern = (c @ adapter_w + adapter_b).reshape(2*CH, CH, K)
    y    = dilated causal conv of x with per-sample kern (K=3, dil=4)
    y   += (c @ bias_w + bias_b)[:, None]
    z    = tanh(y[:CH]) * sigmoid(y[CH:])
    out  = resi_w @ z + resi_b + x
Returns (out, z).

Sharding: data-parallel over batch, 2 samples per core. The two samples are
packed on the 128 SBUF partitions (rows 64b+ch) so every activation / vector
op runs full-width. The per-sample conv kernels are laid out block-diagonally
per (gate, tap) so one matmul computes one gate half for both samples at once.
All matmul operands are bf16 (full PE rate + FWL); accumulation is fp32 PSUM.

DMA: transfers on one HWDGE ring execute FIFO, so the big input loads get a
dedicated ring (sync) while small scatters/constants ride the scalar/vector
rings. Output stores queue on the sync ring behind the loads.

Schedule: tanh-gate adapter chunks load first; a tanh-conv pass runs while the
sigmoid-gate chunks land; the sigmoid+residual pass is interleaved into the
tanh pass emission so the PE stream never barriers between passes.
"""

import numpy as np

from concourse import bacc, mybir, tile
from concourse.bass_utils import run_bass_kernel_spmd

K = 3
DIL = 4
CH = 64
COND = 128
B, T = 16, 16384
NCORES = 8
BL = B // NCORES          # samples per core
PAD = (K - 1) * DIL       # causal left pad = 8
NT = 512                  # matmul free-dim (one PSUM bank of fp32)
UW = 1024                 # processing unit width (2 PSUM banks)
NJ = T // UW
F = K * CH * 2 * CH       # 24576 adapter columns
QCH = CH * CH             # 4096 adapter columns per (gate, tap) block
NXC = 4                   # x load chunks
XC = T // NXC
QLIST = [0, 2, 4, 1, 3, 5]  # chunk order: tanh-gate (even q) blocks first
P1HEAD = 8                # pass-1 tiles emitted before pass-2 interleaving

F32 = mybir.dt.float32
BF16 = mybir.dt.bfloat16
AF = mybir.ActivationFunctionType
ALU = mybir.AluOpType

# Set by test.py to capture a profile; harness path leaves these alone.
TRACE = False
LAST_RESULTS = None

_NC = None


def _build():
    nc = bacc.Bacc("TRN2", target_bir_lowering=False, debug=False)

    x_in = nc.dram_tensor("x_in", [2 * CH, T], BF16, kind="ExternalInput")
    cT_d = nc.dram_tensor("cT", [COND, BL], BF16, kind="ExternalInput")
    aw_d = nc.dram_tensor("aw_r", [COND, F], BF16, kind="ExternalInput")
    ab_d = nc.dram_tensor("ab_p", [2 * CH, 6 * 128], BF16, kind="ExternalInput")
    bw_d = nc.dram_tensor("bw", [COND, 2 * CH], BF16, kind="ExternalInput")
    bbt_d = nc.dram_tensor("bbt", [2 * CH, 1], F32, kind="ExternalInput")
    bbs_d = nc.dram_tensor("bbs", [2 * CH, 1], F32, kind="ExternalInput")
    rwT_d = nc.dram_tensor("rwT", [2 * CH, 2 * CH], BF16, kind="ExternalInput")
    rb_d = nc.dram_tensor("rb", [2 * CH, 1], F32, kind="ExternalInput")
    out_d = nc.dram_tensor("out_d", [NJ, 2 * CH, UW], BF16, kind="ExternalOutput")
    z_d = nc.dram_tensor("z_d", [NJ, 2 * CH, UW], BF16, kind="ExternalOutput")

    with tile.TileContext(nc) as tc:
        with (
            tc.tile_pool(name="const", bufs=1) as constp,
            tc.tile_pool(name="xpool", bufs=1) as xpool,
            tc.tile_pool(name="kern", bufs=1) as kernp,
        ):
            # ---- sync ring: big input loads only, in priority order --------
            cT_sb = constp.tile([COND, BL], BF16)
            nc.sync.dma_start(cT_sb[:, :], cT_d[:, :])
            bw_sb = constp.tile([COND, 2 * CH], BF16)
            nc.sync.dma_start(bw_sb[:, :], bw_d[:, :])

            aw_sb = constp.tile([COND, F], BF16, name="aw_sb")
            xz = xpool.tile([2 * CH, PAD + T], BF16)
            nc.vector.memset(xz[:, 0:PAD].bitcast(F32), 0.0)

            nc.sync.dma_start(xz[:, PAD : PAD + XC], x_in[:, 0:XC])
            nc.sync.dma_start(aw_sb[:, 0 : 3 * QCH], aw_d[:, 0 : 3 * QCH])
            nc.sync.dma_start(
                xz[:, PAD + XC : PAD + 2 * XC], x_in[:, XC : 2 * XC]
            )
            nc.sync.dma_start(aw_sb[:, 3 * QCH : 6 * QCH], aw_d[:, 3 * QCH : 6 * QCH])
            nc.sync.dma_start(
                xz[:, PAD + 2 * XC : PAD + 3 * XC], x_in[:, 2 * XC : 3 * XC]
            )
            nc.sync.dma_start(
                xz[:, PAD + 3 * XC : PAD + 4 * XC], x_in[:, 3 * XC : 4 * XC]
            )

            # ---- scalar ring: small constants ------------------------------
            ab_sb = constp.tile([2 * CH, 6 * 128], BF16)
            nc.scalar.dma_start(ab_sb[:, :], ab_d[:, :])
            rwT_sb = constp.tile([2 * CH, 2 * CH], BF16)
            nc.scalar.dma_start(rwT_sb[:, :], rwT_d[:, :])
            rb_sb = constp.tile([2 * CH, 1], F32)
            nc.scalar.dma_start(rb_sb[:, :], rb_d[:, :])
            bbt_sb = constp.tile([2 * CH, 1], F32)
            nc.scalar.dma_start(bbt_sb[:, :], bbt_d[:, :])
            bbs_sb = constp.tile([2 * CH, 1], F32)
            nc.scalar.dma_start(bbs_sb[:, :], bbs_d[:, :])

            # Block-diagonal per-(gate,tap) kernel tiles: block q=2k+g holds
            # lhsT[64b+i, 64b+o'] = kern[b, g*64+o', i, k].
            kern_raw = kernp.tile([2 * CH, 6 * 128], BF16, name="kern_raw")
            nc.vector.memset(kern_raw[:, :].bitcast(F32), 0.0)
            kern = kernp.tile([2 * CH, 6 * 128], BF16, name="kern")
            bias_t = kernp.tile([2 * CH, 1], F32)
            bias_s = kernp.tile([2 * CH, 1], F32)
            # tanh-gate activations for all tiles (pass 1 output)
            ta_all = xpool.tile([2 * CH, T], BF16, name="ta_all")

            # ---------------- phase A: conditioned bias ---------------------
            with (
                tc.tile_pool(name="bps", bufs=1, space="PSUM") as bpsp,
                tc.tile_pool(name="bstg", bufs=1) as bstgp,
            ):
                pb = bpsp.tile([2 * CH, BL], F32)
                nc.tensor.matmul(pb[:, :], bw_sb[:, :], cT_sb[:, :], start=True, stop=True)
                pbs = bstgp.tile([2 * CH, BL], F32)
                nc.vector.tensor_copy(pbs[:, :], pb[:, :])
                # pair layout: rows 64b+o' = bias for sample b, out-chan o'
                nc.scalar.dma_start(bias_t[0:CH, :], pbs[0:CH, 0:1])
                nc.scalar.dma_start(bias_t[CH : 2 * CH, :], pbs[0:CH, 1:2])
                nc.scalar.dma_start(bias_s[0:CH, :], pbs[CH : 2 * CH, 0:1])
                nc.scalar.dma_start(bias_s[CH : 2 * CH, :], pbs[CH : 2 * CH, 1:2])
                nc.vector.tensor_add(bias_t[:, :], bias_t[:, :], bbt_sb[:, :])
                nc.vector.tensor_add(bias_s[:, :], bias_s[:, :], bbs_sb[:, :])

            # ------------- adapter chunks + two interleaved conv passes -----
            with (
                tc.tile_pool(name="stg", bufs=3) as stgp,
                tc.tile_pool(name="work", bufs=3) as workp,
            ):
                pools = {}

                def adapter_chunk(cidx, late):
                    """late chunks use the vector engine + gpsimd ring (idle in pass 1)."""
                    q = QLIST[cidx]
                    awt = aw_sb[:, cidx * QCH : (cidx + 1) * QCH]
                    for h2 in range(2):
                        ps = pools["apsum"].tile([BL, 2048], F32, tag="aps")
                        for v in range(4):
                            nc.tensor.matmul(
                                ps[:, 512 * v : 512 * (v + 1)],
                                cT_sb[:, :],
                                awt[:, 2048 * h2 + 512 * v : 2048 * h2 + 512 * (v + 1)],
                                start=True,
                                stop=True,
                            )
                        stg = stgp.tile([BL, 2048], BF16, tag="stg")
                        if late:
                            nc.vector.tensor_copy(stg[:, :], ps[:, :])
                        else:
                            nc.scalar.activation(stg[:, 0:1024], ps[:, 0:1024], AF.Copy)
                            nc.vector.tensor_copy(stg[:, 1024:2048], ps[:, 1024:2048])
                        dma_eng = nc.gpsimd if late else nc.scalar
                        for b in range(BL):
                            dma_eng.dma_start(
                                kern_raw[
                                    CH * b + 32 * h2 : CH * b + 32 * h2 + 32,
                                    128 * q + CH * b : 128 * q + CH * b + CH,
                                ],
                                stg[b : b + 1, :],
                            )
                    nc.vector.tensor_add(
                        kern[:, 128 * q : 128 * (q + 1)],
                        kern_raw[:, 128 * q : 128 * (q + 1)],
                        ab_sb[:, 128 * q : 128 * (q + 1)],
                    )

                def pass1_tile(j):
                    pt = pools["tpsum"].tile([2 * CH, UW], F32, tag="pt")
                    for k in range(K):
                        q = 2 * k
                        for h in range(UW // NT):
                            c0 = j * UW + h * NT + DIL * k
                            nc.tensor.matmul(
                                pt[:, h * NT : (h + 1) * NT],
                                kern[:, 128 * q : 128 * (q + 1)],
                                xz[:, c0 : c0 + NT],
                                start=(k == 0),
                                stop=(k == K - 1),
                            )
                    nc.scalar.activation(
                        ta_all[:, j * UW : (j + 1) * UW],
                        pt[:, :],
                        AF.Tanh,
                        bias=bias_t[:, 0:1],
                    )

                def emit_residual(j, zz):
                    po = pools["opsum"].tile([2 * CH, UW], F32, tag="po")
                    for h in range(UW // NT):
                        nc.tensor.matmul(
                            po[:, h * NT : (h + 1) * NT],
                            rwT_sb[:, :],
                            zz[:, h * NT : (h + 1) * NT],
                            start=True,
                            stop=True,
                        )
                    ot = workp.tile([2 * CH, UW], BF16, tag="ot")
                    nc.vector.scalar_tensor_tensor(
                        ot[:, :],
                        po[:, :],
                        rb_sb[:, 0:1],
                        xz[:, j * UW + PAD : j * UW + PAD + UW],
                        ALU.add,
                        ALU.add,
                    )
                    nc.sync.dma_start(out_d[j], ot[:, :])

                state = {"prev": None}

                def pass2_tile(j):
                    psg = pools["spsum"].tile([2 * CH, UW], F32, tag="ps")
                    for k in range(K):
                        q = 2 * k + 1
                        for h in range(UW // NT):
                            c0 = j * UW + h * NT + DIL * k
                            nc.tensor.matmul(
                                psg[:, h * NT : (h + 1) * NT],
                                kern[:, 128 * q : 128 * (q + 1)],
                                xz[:, c0 : c0 + NT],
                                start=(k == 0),
                                stop=(k == K - 1),
                            )
                    if state["prev"] is not None:
                        emit_residual(*state["prev"])
                    ts = workp.tile([2 * CH, UW], BF16, tag="ts")
                    nc.scalar.activation(
                        ts[:, :], psg[:, :], AF.Sigmoid, bias=bias_s[:, 0:1]
                    )
                    zz = workp.tile([2 * CH, UW], BF16, tag="zz")
                    nc.vector.tensor_mul(
                        zz[:, 0:NT], ta_all[:, j * UW : j * UW + NT], ts[:, 0:NT]
                    )
                    nc.gpsimd.tensor_mul(
                        zz[:, NT:UW], ta_all[:, j * UW + NT : (j + 1) * UW], ts[:, NT:UW]
                    )
                    nc.sync.dma_start(z_d[j], zz[:, :])
                    state["prev"] = (j, zz)

                # scope 1: adapter PSUM + pass-1 pipeline (8 banks)
                with (
                    tc.tile_pool(name="apsum", bufs=1, space="PSUM") as apsum,
                    tc.tile_pool(name="tpsum", bufs=2, space="PSUM") as tpsum,
                ):
                    pools["apsum"], pools["tpsum"] = apsum, tpsum
                    for cidx in range(3):
                        adapter_chunk(cidx, late=False)
                    for j in range(P1HEAD):
                        pass1_tile(j)
                    for cidx in range(3, 6):
                        adapter_chunk(cidx, late=True)
                # scope 2: interleaved pass-1 tail + pass-2 (2+4+2 banks)
                with (
                    tc.tile_pool(name="tpsum2", bufs=1, space="PSUM") as tpsum2,
                    tc.tile_pool(name="spsum", bufs=2, space="PSUM") as spsum,
                    tc.tile_pool(name="opsum", bufs=1, space="PSUM") as opsum,
                ):
                    pools["tpsum"], pools["spsum"], pools["opsum"] = tpsum2, spsum, opsum
                    for j in range(P1HEAD, NJ + P1HEAD):
                        if j < NJ:
                            pass1_tile(j)
                        pass2_tile(j - P1HEAD)
                    emit_residual(*state["prev"])

    nc.compile()
    return nc


def get_nc():
    global _NC
    if _NC is None:
        _NC = _build()
    return _NC


def make_in_maps(inputs):
    import ml_dtypes

    bf = ml_dtypes.bfloat16

    x = np.asarray(inputs["x"], np.float32)
    c = np.asarray(inputs["c"], np.float32)
    aw = np.asarray(inputs["adapter_w"], np.float32)
    ab = np.asarray(inputs["adapter_b"], np.float32)
    bw = np.asarray(inputs["bias_w"], np.float32)
    bb = np.asarray(inputs["bias_b"], np.float32)
    rw = np.asarray(inputs["resi_w"], np.float32)
    rb = np.asarray(inputs["resi_b"], np.float32)

    # adapter cols [cond, (g,o',i,k)] -> chunks of (i, o') per q=2k+g in QLIST order
    aw4 = aw.reshape(COND, 2, CH, CH, K)
    chunks = []
    for cidx in range(6):
        q = QLIST[cidx]
        g, k = q % 2, q // 2
        blk = aw4[:, g, :, :, k]  # [cond, o', i]
        chunks.append(np.ascontiguousarray(blk.transpose(0, 2, 1)).reshape(COND, QCH))
    aw_r = np.ascontiguousarray(np.concatenate(chunks, axis=1).astype(bf))

    # adapter bias in the block-diagonal pair layout (zeros off-diagonal)
    ab4 = ab.reshape(2, CH, CH, K)
    ab_p = np.zeros((2 * CH, 6 * 128), np.float32)
    for q in range(6):
        g, k = q % 2, q // 2
        blk = ab4[g, :, :, k].T  # [i, o']
        for b2 in range(BL):
            ab_p[CH * b2 : CH * (b2 + 1), 128 * q + CH * b2 : 128 * q + CH * b2 + CH] = blk
    ab_p = np.ascontiguousarray(ab_p.astype(bf))

    rwT_p = np.zeros((2 * CH, 2 * CH), np.float32)
    rwT_p[0:CH, 0:CH] = rw.T
    rwT_p[CH:, CH:] = rw.T
    rwT_p = np.ascontiguousarray(rwT_p.astype(bf))

    bbt = np.ascontiguousarray(np.tile(bb[0:CH], 2).reshape(2 * CH, 1))
    bbs = np.ascontiguousarray(np.tile(bb[CH:], 2).reshape(2 * CH, 1))
    rbp = np.ascontiguousarray(np.tile(rb, 2).reshape(2 * CH, 1))
    bw_b = np.ascontiguousarray(bw.astype(bf))

    in_maps = []
    for m in range(NCORES):
        sl = slice(BL * m, BL * (m + 1))
        in_maps.append(
            {
                "x_in": np.ascontiguousarray(x[sl].reshape(2 * CH, T).astype(bf)),
                "cT": np.ascontiguousarray(c[sl].T.astype(bf)),
                "aw_r": aw_r,
                "ab_p": ab_p,
                "bw": bw_b,
                "bbt": bbt,
                "bbs": bbs,
                "rwT": rwT_p,
                "rb": rbp,
            }
        )
    return in_maps


def kernel(**inputs):
    global LAST_RESULTS
    nc = get_nc()
    in_maps = make_in_maps(inputs)
    res = run_bass_kernel_spmd(nc, in_maps, list(range(NCORES)), trace=TRACE)
    LAST_RESULTS = res
    out = np.empty((B, CH, T), np.float32)
    z = np.empty((B, CH, T), np.float32)
    for m in range(NCORES):
        # [NJ, 128, UW] -> [2, CH, T]
        o = res.results[m]["out_d"].astype(np.float32)
        zt = res.results[m]["z_d"].astype(np.float32)
        out[BL * m : BL * (m + 1)] = o.transpose(1, 0, 2).reshape(BL, CH, T)
        z[BL * m : BL * (m + 1)] = zt.transpose(1, 0, 2).reshape(BL, CH, T)
    return out, z


# revision 21
# speedup vs baseline: 1.0929x; 1.0328x over previous
"""Gated TCN layer (fully conditioned) as a Bass/Tile kernel on 8 NeuronCores.

Reference computation (per sample b):
    kern = (c @ adapter_w + adapter_b).reshape(2*CH, CH, K)
    y    = dilated causal conv of x with per-sample kern (K=3, dil=4)
    y   += (c @ bias_w + bias_b)[:, None]
    z    = tanh(y[:CH]) * sigmoid(y[CH:])
    out  = resi_w @ z + resi_b + x
Returns (out, z).

Sharding: data-parallel over batch, 2 samples per core. The two samples are
packed on the 128 SBUF partitions (rows 64b+ch) so every activation / vector
op runs full-width. The per-sample conv kernels are laid out block-diagonally
per (gate, tap) so one matmul computes one gate half for both samples at once.
All matmul operands are bf16 (full PE rate + FWL); accumulation is fp32 PSUM.

DMA: transfers on one HWDGE ring execute FIFO, so the big input loads get a
dedicated ring (sync) while small scatters/constants ride the scalar/vector
rings. Output stores queue on the sync ring behind the loads.

Schedule: tanh-gate adapter chunks load first; a tanh-conv pass runs while the
sigmoid-gate chunks land; the sigmoid+residual pass is interleaved into the
tanh pass emission so the PE stream never barriers between passes.
"""

import numpy as np

from concourse import bacc, mybir, tile
from concourse.bass_utils import run_bass_kernel_spmd

K = 3
DIL = 4
CH = 64
COND = 128
B, T = 16, 16384
NCORES = 8
BL = B // NCORES          # samples per core
PAD = (K - 1) * DIL       # causal left pad = 8
NT = 512                  # matmul free-dim (one PSUM bank of fp32)
UW = 1024                 # processing unit width (2 PSUM banks)
NJ = T // UW
F = K * CH * 2 * CH       # 24576 adapter columns
QCH = CH * CH             # 4096 adapter columns per (gate, tap) block
NXC = 4                   # x load chunks
XC = T // NXC
QLIST = [0, 2, 4, 1, 3, 5]  # chunk order: tanh-gate (even q) blocks first
P1HEAD = 8                # pass-1 tiles emitted before pass-2 interleaving

F32 = mybir.dt.float32
BF16 = mybir.dt.bfloat16
AF = mybir.ActivationFunctionType
ALU = mybir.AluOpType

# Set by test.py to capture a profile; harness path leaves these alone.
TRACE = False
LAST_RESULTS = None

_NC = None


def _build():
    nc = bacc.Bacc("TRN2", target_bir_lowering=False, debug=False)

    x_in = nc.dram_tensor("x_in", [2 * CH, T], BF16, kind="ExternalInput")
    cT_d = nc.dram_tensor("cT", [COND, BL], BF16, kind="ExternalInput")
    aw_d = nc.dram_tensor("aw_r", [COND, F], BF16, kind="ExternalInput")
    ab_d = nc.dram_tensor("ab_p", [2 * CH, 6 * 128], BF16, kind="ExternalInput")
    bw_d = nc.dram_tensor("bw", [COND, 2 * CH], BF16, kind="ExternalInput")
    bbt_d = nc.dram_tensor("bbt", [2 * CH, 1], F32, kind="ExternalInput")
    bbs_d = nc.dram_tensor("bbs", [2 * CH, 1], F32, kind="ExternalInput")
    rwT_d = nc.dram_tensor("rwT", [2 * CH, 2 * CH], BF16, kind="ExternalInput")
    rb_d = nc.dram_tensor("rb", [2 * CH, 1], F32, kind="ExternalInput")
    out_d = nc.dram_tensor("out_d", [NJ, 2 * CH, UW], BF16, kind="ExternalOutput")
    z_d = nc.dram_tensor("z_d", [NJ, 2 * CH, UW], BF16, kind="ExternalOutput")

    with tile.TileContext(nc) as tc:
        with (
            tc.tile_pool(name="const", bufs=1) as constp,
            tc.tile_pool(name="xpool", bufs=1) as xpool,
            tc.tile_pool(name="kern", bufs=1) as kernp,
        ):
            # ---- sync ring: big input loads only, in priority order --------
            cT_sb = constp.tile([COND, BL], BF16)
            nc.sync.dma_start(cT_sb[:, :], cT_d[:, :])
            bw_sb = constp.tile([COND, 2 * CH], BF16)
            nc.sync.dma_start(bw_sb[:, :], bw_d[:, :])

            aw_sb = constp.tile([COND, F], BF16, name="aw_sb")
            xz = xpool.tile([2 * CH, PAD + T], BF16)
            nc.vector.memset(xz[:, 0:PAD].bitcast(F32), 0.0)

            nc.sync.dma_start(aw_sb[:, 0 : 3 * QCH], aw_d[:, 0 : 3 * QCH])
            nc.sync.dma_start(xz[:, PAD : PAD + XC], x_in[:, 0:XC])
            nc.sync.dma_start(
                xz[:, PAD + XC : PAD + 2 * XC], x_in[:, XC : 2 * XC]
            )
            nc.sync.dma_start(aw_sb[:, 3 * QCH : 6 * QCH], aw_d[:, 3 * QCH : 6 * QCH])
            nc.sync.dma_start(
                xz[:, PAD + 2 * XC : PAD + 3 * XC], x_in[:, 2 * XC : 3 * XC]
            )
            nc.sync.dma_start(
                xz[:, PAD + 3 * XC : PAD + 4 * XC], x_in[:, 3 * XC : 4 * XC]
            )

            # ---- scalar ring: small constants ------------------------------
            ab_sb = constp.tile([2 * CH, 6 * 128], BF16)
            nc.scalar.dma_start(ab_sb[:, :], ab_d[:, :])
            rwT_sb = constp.tile([2 * CH, 2 * CH], BF16)
            nc.scalar.dma_start(rwT_sb[:, :], rwT_d[:, :])
            rb_sb = constp.tile([2 * CH, 1], F32)
            nc.scalar.dma_start(rb_sb[:, :], rb_d[:, :])
            bbt_sb = constp.tile([2 * CH, 1], F32)
            nc.scalar.dma_start(bbt_sb[:, :], bbt_d[:, :])
            bbs_sb = constp.tile([2 * CH, 1], F32)
            nc.scalar.dma_start(bbs_sb[:, :], bbs_d[:, :])

            # Block-diagonal per-(gate,tap) kernel tiles: block q=2k+g holds
            # lhsT[64b+i, 64b+o'] = kern[b, g*64+o', i, k].
            kern_raw = kernp.tile([2 * CH, 6 * 128], BF16, name="kern_raw")
            nc.vector.memset(kern_raw[:, :].bitcast(F32), 0.0)
            kern = kernp.tile([2 * CH, 6 * 128], BF16, name="kern")
            bias_t = kernp.tile([2 * CH, 1], F32)
            bias_s = kernp.tile([2 * CH, 1], F32)
            # tanh-gate activations for all tiles (pass 1 output)
            ta_all = xpool.tile([2 * CH, T], BF16, name="ta_all")

            # ---------------- phase A: conditioned bias ---------------------
            with (
                tc.tile_pool(name="bps", bufs=1, space="PSUM") as bpsp,
                tc.tile_pool(name="bstg", bufs=1) as bstgp,
            ):
                pb = bpsp.tile([2 * CH, BL], F32)
                nc.tensor.matmul(pb[:, :], bw_sb[:, :], cT_sb[:, :], start=True, stop=True)
                pbs = bstgp.tile([2 * CH, BL], F32)
                nc.vector.tensor_copy(pbs[:, :], pb[:, :])
                # pair layout: rows 64b+o' = bias for sample b, out-chan o'
                nc.scalar.dma_start(bias_t[0:CH, :], pbs[0:CH, 0:1])
                nc.scalar.dma_start(bias_t[CH : 2 * CH, :], pbs[0:CH, 1:2])
                nc.scalar.dma_start(bias_s[0:CH, :], pbs[CH : 2 * CH, 0:1])
                nc.scalar.dma_start(bias_s[CH : 2 * CH, :], pbs[CH : 2 * CH, 1:2])
                nc.vector.tensor_add(bias_t[:, :], bias_t[:, :], bbt_sb[:, :])
                nc.vector.tensor_add(bias_s[:, :], bias_s[:, :], bbs_sb[:, :])

            # ------------- adapter chunks + two interleaved conv passes -----
            with (
                tc.tile_pool(name="stg", bufs=3) as stgp,
                tc.tile_pool(name="work", bufs=3) as workp,
            ):
                pools = {}

                def adapter_chunk(cidx, late):
                    """late chunks use the vector engine + gpsimd ring (idle in pass 1)."""
                    q = QLIST[cidx]
                    awt = aw_sb[:, cidx * QCH : (cidx + 1) * QCH]
                    for h2 in range(2):
                        ps = pools["apsum"].tile([BL, 2048], F32, tag="aps")
                        for v in range(4):
                            nc.tensor.matmul(
                                ps[:, 512 * v : 512 * (v + 1)],
                                cT_sb[:, :],
                                awt[:, 2048 * h2 + 512 * v : 2048 * h2 + 512 * (v + 1)],
                                start=True,
                                stop=True,
                            )
                        stg = stgp.tile([BL, 2048], BF16, tag="stg")
                        if late:
                            nc.vector.tensor_copy(stg[:, :], ps[:, :])
                        else:
                            nc.scalar.activation(stg[:, 0:1024], ps[:, 0:1024], AF.Copy)
                            nc.vector.tensor_copy(stg[:, 1024:2048], ps[:, 1024:2048])
                        dma_eng = nc.sync if late else nc.scalar
                        for b in range(BL):
                            dma_eng.dma_start(
                                kern_raw[
                                    CH * b + 32 * h2 : CH * b + 32 * h2 + 32,
                                    128 * q + CH * b : 128 * q + CH * b + CH,
                                ],
                                stg[b : b + 1, :],
                            )
                    nc.vector.tensor_add(
                        kern[:, 128 * q : 128 * (q + 1)],
                        kern_raw[:, 128 * q : 128 * (q + 1)],
                        ab_sb[:, 128 * q : 128 * (q + 1)],
                    )

                def pass1_tile(j):
                    pt = pools["tpsum"].tile([2 * CH, UW], F32, tag="pt")
                    for k in range(K):
                        q = 2 * k
                        for h in range(UW // NT):
                            c0 = j * UW + h * NT + DIL * k
                            nc.tensor.matmul(
                                pt[:, h * NT : (h + 1) * NT],
                                kern[:, 128 * q : 128 * (q + 1)],
                                xz[:, c0 : c0 + NT],
                                start=(k == 0),
                                stop=(k == K - 1),
                            )
                    nc.scalar.activation(
                        ta_all[:, j * UW : (j + 1) * UW],
                        pt[:, :],
                        AF.Tanh,
                        bias=bias_t[:, 0:1],
                    )

                def emit_residual(j, zz):
                    po = pools["opsum"].tile([2 * CH, UW], F32, tag="po")
                    for h in range(UW // NT):
                        nc.tensor.matmul(
                            po[:, h * NT : (h + 1) * NT],
                            rwT_sb[:, :],
                            zz[:, h * NT : (h + 1) * NT],
                            start=True,
                            stop=True,
                        )
                    ot = workp.tile([2 * CH, UW], BF16, tag="ot")
                    nc.vector.scalar_tensor_tensor(
                        ot[:, :],
                        po[:, :],
                        rb_sb[:, 0:1],
                        xz[:, j * UW + PAD : j * UW + PAD + UW],
                        ALU.add,
                        ALU.add,
                    )
                    nc.sync.dma_start(out_d[j], ot[:, :])

                state = {"prev": None}

                def pass2_tile(j):
                    psg = pools["spsum"].tile([2 * CH, UW], F32, tag="ps")
                    for k in range(K):
                        q = 2 * k + 1
                        for h in range(UW // NT):
                            c0 = j * UW + h * NT + DIL * k
                            nc.tensor.matmul(
                                psg[:, h * NT : (h + 1) * NT],
                                kern[:, 128 * q : 128 * (q + 1)],
                                xz[:, c0 : c0 + NT],
                                start=(k == 0),
                                stop=(k == K - 1),
                            )
                    if state["prev"] is not None:
                        emit_residual(*state["prev"])
                    ts = workp.tile([2 * CH, UW], BF16, tag="ts")
                    nc.scalar.activation(
                        ts[:, :], psg[:, :], AF.Sigmoid, bias=bias_s[:, 0:1]
                    )
                    zz = workp.tile([2 * CH, UW], BF16, tag="zz")
                    nc.vector.tensor_mul(
                        zz[:, :], ta_all[:, j * UW : (j + 1) * UW], ts[:, :]
                    )
                    nc.sync.dma_start(z_d[j], zz[:, :])
                    state["prev"] = (j, zz)

                # scope 1: adapter PSUM + pass-1 pipeline (8 banks)
                with (
                    tc.tile_pool(name="apsum", bufs=1, space="PSUM") as apsum,
                    tc.tile_pool(name="tpsum", bufs=2, space="PSUM") as tpsum,
                ):
                    pools["apsum"], pools["tpsum"] = apsum, tpsum
                    for cidx in range(3):
                        adapter_chunk(cidx, late=False)
                    for j in range(P1HEAD):
                        pass1_tile(j)
                    for cidx in range(3, 6):
                        adapter_chunk(cidx, late=True)
                # scope 2: interleaved pass-1 tail + pass-2 (2+4+2 banks)
                with (
                    tc.tile_pool(name="tpsum2", bufs=1, space="PSUM") as tpsum2,
                    tc.tile_pool(name="spsum", bufs=2, space="PSUM") as spsum,
                    tc.tile_pool(name="opsum", bufs=1, space="PSUM") as opsum,
                ):
                    pools["tpsum"], pools["spsum"], pools["opsum"] = tpsum2, spsum, opsum
                    for j in range(P1HEAD, NJ + P1HEAD):
                        if j < NJ:
                            pass1_tile(j)
                        pass2_tile(j - P1HEAD)
                    emit_residual(*state["prev"])

    nc.compile()
    return nc


def get_nc():
    global _NC
    if _NC is None:
        _NC = _build()
    return _NC


def make_in_maps(inputs):
    import ml_dtypes

    bf = ml_dtypes.bfloat16

    x = np.asarray(inputs["x"], np.float32)
    c = np.asarray(inputs["c"], np.float32)
    aw = np.asarray(inputs["adapter_w"], np.float32)
    ab = np.asarray(inputs["adapter_b"], np.float32)
    bw = np.asarray(inputs["bias_w"], np.float32)
    bb = np.asarray(inputs["bias_b"], np.float32)
    rw = np.asarray(inputs["resi_w"], np.float32)
    rb = np.asarray(inputs["resi_b"], np.float32)

    # adapter cols [cond, (g,o',i,k)] -> chunks of (i, o') per q=2k+g in QLIST order
    aw4 = aw.reshape(COND, 2, CH, CH, K)
    chunks = []
    for cidx in range(6):
        q = QLIST[cidx]
        g, k = q % 2, q // 2
        blk = aw4[:, g, :, :, k]  # [cond, o', i]
        chunks.append(np.ascontiguousarray(blk.transpose(0, 2, 1)).reshape(COND, QCH))
    aw_r = np.ascontiguousarray(np.concatenate(chunks, axis=1).astype(bf))

    # adapter bias in the block-diagonal pair layout (zeros off-diagonal)
    ab4 = ab.reshape(2, CH, CH, K)
    ab_p = np.zeros((2 * CH, 6 * 128), np.float32)
    for q in range(6):
        g, k = q % 2, q // 2
        blk = ab4[g, :, :, k].T  # [i, o']
        for b2 in range(BL):
            ab_p[CH * b2 : CH * (b2 + 1), 128 * q + CH * b2 : 128 * q + CH * b2 + CH] = blk
    ab_p = np.ascontiguousarray(ab_p.astype(bf))

    rwT_p = np.zeros((2 * CH, 2 * CH), np.float32)
    rwT_p[0:CH, 0:CH] = rw.T
    rwT_p[CH:, CH:] = rw.T
    rwT_p = np.ascontiguousarray(rwT_p.astype(bf))

    bbt = np.ascontiguousarray(np.tile(bb[0:CH], 2).reshape(2 * CH, 1))
    bbs = np.ascontiguousarray(np.tile(bb[CH:], 2).reshape(2 * CH, 1))
    rbp = np.ascontiguousarray(np.tile(rb, 2).reshape(2 * CH, 1))
    bw_b = np.ascontiguousarray(bw.astype(bf))

    in_maps = []
    for m in range(NCORES):
        sl = slice(BL * m, BL * (m + 1))
        in_maps.append(
            {
                "x_in": np.ascontiguousarray(x[sl].reshape(2 * CH, T).astype(bf)),
                "cT": np.ascontiguousarray(c[sl].T.astype(bf)),
                "aw_r": aw_r,
                "ab_p": ab_p,
                "bw": bw_b,
                "bbt": bbt,
                "bbs": bbs,
                "rwT": rwT_p,
                "rb": rbp,
            }
        )
    return in_maps


def kernel(**inputs):
    global LAST_RESULTS
    nc = get_nc()
    in_maps = make_in_maps(inputs)
    res = run_bass_kernel_spmd(nc, in_maps, list(range(NCORES)), trace=TRACE)
    LAST_RESULTS = res
    out = np.empty((B, CH, T), np.float32)
    z = np.empty((B, CH, T), np.float32)
    for m in range(NCORES):
        # [NJ, 128, UW] -> [2, CH, T]
        o = res.results[m]["out_d"].astype(np.float32)
        zt = res.results[m]["z_d"].astype(np.float32)
        out[BL * m : BL * (m + 1)] = o.transpose(1, 0, 2).reshape(BL, CH, T)
        z[BL * m : BL * (m + 1)] = zt.transpose(1, 0, 2).reshape(BL, CH, T)
    return out, z


# revision 24
# speedup vs baseline: 1.1113x; 1.0168x over previous
"""Gated TCN layer (fully conditioned) as a Bass/Tile kernel on 8 NeuronCores.

Reference computation (per sample b):
    kern = (c @ adapter_w + adapter_b).reshape(2*CH, CH, K)
    y    = dilated causal conv of x with per-sample kern (K=3, dil=4)
    y   += (c @ bias_w + bias_b)[:, None]
    z    = tanh(y[:CH]) * sigmoid(y[CH:])
    out  = resi_w @ z + resi_b + x
Returns (out, z).

Sharding: data-parallel over batch, 2 samples per core. The two samples are
packed on the 128 SBUF partitions (rows 64b+ch) so every activation / vector
op runs full-width. The per-sample conv kernels are laid out block-diagonally
per (gate, tap) so one matmul computes one gate half for both samples at once.
All matmul operands are bf16 (full PE rate + FWL); accumulation is fp32 PSUM.

DMA: transfers on one HWDGE ring execute FIFO, so the big input loads get a
dedicated ring (sync) while small scatters/constants ride the scalar/vector
rings. Output stores queue on the sync ring behind the loads.

Schedule: tanh-gate adapter chunks load first; a tanh-conv pass runs while the
sigmoid-gate chunks land; the sigmoid+residual pass is interleaved into the
tanh pass emission so the PE stream never barriers between passes.
"""

import numpy as np

from concourse import bacc, mybir, tile
from concourse.bass_utils import run_bass_kernel_spmd

K = 3
DIL = 4
CH = 64
COND = 128
B, T = 16, 16384
NCORES = 8
BL = B // NCORES          # samples per core
PAD = (K - 1) * DIL       # causal left pad = 8
NT = 512                  # matmul free-dim (one PSUM bank of fp32)
UW = 1024                 # processing unit width (2 PSUM banks)
NJ = T // UW
F = K * CH * 2 * CH       # 24576 adapter columns
QCH = CH * CH             # 4096 adapter columns per (gate, tap) block
NXC = 4                   # x load chunks
XC = T // NXC
QLIST = [0, 2, 4, 1, 3, 5]  # chunk order: tanh-gate (even q) blocks first
P1HEAD = 8                # pass-1 tiles emitted before pass-2 interleaving

F32 = mybir.dt.float32
BF16 = mybir.dt.bfloat16
AF = mybir.ActivationFunctionType
ALU = mybir.AluOpType

# Set by test.py to capture a profile; harness path leaves these alone.
TRACE = False
LAST_RESULTS = None

_NC = None


def _build():
    nc = bacc.Bacc("TRN2", target_bir_lowering=False, debug=False)

    x_in = nc.dram_tensor("x_in", [2 * CH, T], BF16, kind="ExternalInput")
    cT_d = nc.dram_tensor("cT", [COND, BL], BF16, kind="ExternalInput")
    aw_d = nc.dram_tensor("aw_r", [COND, F], BF16, kind="ExternalInput")
    ab_d = nc.dram_tensor("ab_p", [2 * CH, 6 * 128], BF16, kind="ExternalInput")
    bw_d = nc.dram_tensor("bw", [COND, 2 * CH], BF16, kind="ExternalInput")
    bbt_d = nc.dram_tensor("bbt", [2 * CH, 1], F32, kind="ExternalInput")
    bbs_d = nc.dram_tensor("bbs", [2 * CH, 1], F32, kind="ExternalInput")
    rwT_d = nc.dram_tensor("rwT", [2 * CH, 2 * CH], BF16, kind="ExternalInput")
    rb_d = nc.dram_tensor("rb", [2 * CH, 1], F32, kind="ExternalInput")
    out_d = nc.dram_tensor("out_d", [NJ, 2 * CH, UW], BF16, kind="ExternalOutput")
    z_d = nc.dram_tensor("z_d", [NJ, 2 * CH, UW], BF16, kind="ExternalOutput")

    with tile.TileContext(nc) as tc:
        with (
            tc.tile_pool(name="const", bufs=1) as constp,
            tc.tile_pool(name="xpool", bufs=1) as xpool,
            tc.tile_pool(name="kern", bufs=1) as kernp,
        ):
            # ---- sync ring: big input loads only, in priority order --------
            cT_sb = constp.tile([COND, BL], BF16)
            nc.sync.dma_start(cT_sb[:, :], cT_d[:, :])
            bw_sb = constp.tile([COND, 2 * CH], BF16)
            nc.sync.dma_start(bw_sb[:, :], bw_d[:, :])

            aw_sb = constp.tile([COND, F], BF16, name="aw_sb")
            xz = xpool.tile([2 * CH, PAD + T], BF16)
            nc.vector.memset(xz[:, 0:PAD].bitcast(F32), 0.0)

            nc.sync.dma_start(aw_sb[:, 0 : 3 * QCH], aw_d[:, 0 : 3 * QCH])
            nc.sync.dma_start(aw_sb[:, 3 * QCH : 6 * QCH], aw_d[:, 3 * QCH : 6 * QCH])
            for cx in range(NXC):
                nc.sync.dma_start(
                    xz[:, PAD + cx * XC : PAD + (cx + 1) * XC],
                    x_in[:, cx * XC : (cx + 1) * XC],
                )

            # ---- scalar ring: small constants ------------------------------
            ab_sb = constp.tile([2 * CH, 6 * 128], BF16)
            nc.scalar.dma_start(ab_sb[:, :], ab_d[:, :])
            rwT_sb = constp.tile([2 * CH, 2 * CH], BF16)
            nc.scalar.dma_start(rwT_sb[:, :], rwT_d[:, :])
            rb_sb = constp.tile([2 * CH, 1], F32)
            nc.scalar.dma_start(rb_sb[:, :], rb_d[:, :])
            bbt_sb = constp.tile([2 * CH, 1], F32)
            nc.scalar.dma_start(bbt_sb[:, :], bbt_d[:, :])
            bbs_sb = constp.tile([2 * CH, 1], F32)
            nc.scalar.dma_start(bbs_sb[:, :], bbs_d[:, :])

            # Block-diagonal per-(gate,tap) kernel tiles: block q=2k+g holds
            # lhsT[64b+i, 64b+o'] = kern[b, g*64+o', i, k].
            kern_raw = kernp.tile([2 * CH, 6 * 128], BF16, name="kern_raw")
            nc.vector.memset(kern_raw[:, :].bitcast(F32), 0.0)
            kern = kernp.tile([2 * CH, 6 * 128], BF16, name="kern")
            bias_t = kernp.tile([2 * CH, 1], F32)
            bias_s = kernp.tile([2 * CH, 1], F32)

            # ---------------- phase A: conditioned bias ---------------------
            with (
                tc.tile_pool(name="bps", bufs=1, space="PSUM") as bpsp,
                tc.tile_pool(name="bstg", bufs=1) as bstgp,
            ):
                pb = bpsp.tile([2 * CH, BL], F32)
                nc.tensor.matmul(pb[:, :], bw_sb[:, :], cT_sb[:, :], start=True, stop=True)
                pbs = bstgp.tile([2 * CH, BL], F32)
                nc.vector.tensor_copy(pbs[:, :], pb[:, :])
                # pair layout: rows 64b+o' = bias for sample b, out-chan o'
                nc.scalar.dma_start(bias_t[0:CH, :], pbs[0:CH, 0:1])
                nc.scalar.dma_start(bias_t[CH : 2 * CH, :], pbs[0:CH, 1:2])
                nc.scalar.dma_start(bias_s[0:CH, :], pbs[CH : 2 * CH, 0:1])
                nc.scalar.dma_start(bias_s[CH : 2 * CH, :], pbs[CH : 2 * CH, 1:2])
                nc.vector.tensor_add(bias_t[:, :], bias_t[:, :], bbt_sb[:, :])
                nc.vector.tensor_add(bias_s[:, :], bias_s[:, :], bbs_sb[:, :])

            # ---------------- phase A: adapter -> dynamic kernels -----------
            with (
                tc.tile_pool(name="apsum", bufs=2, space="PSUM") as apsum,
                tc.tile_pool(name="stg", bufs=3) as stgp,
            ):
                for cidx in range(6):
                    q = QLIST[cidx]
                    awt = aw_sb[:, cidx * QCH : (cidx + 1) * QCH]
                    for h2 in range(2):
                        ps = apsum.tile([BL, 2048], F32, tag="aps")
                        for v in range(4):
                            nc.tensor.matmul(
                                ps[:, 512 * v : 512 * (v + 1)],
                                cT_sb[:, :],
                                awt[:, 2048 * h2 + 512 * v : 2048 * h2 + 512 * (v + 1)],
                                start=True,
                                stop=True,
                            )
                        # drain PSUM with scalar and vector in parallel halves
                        stg = stgp.tile([BL, 2048], BF16, tag="stg")
                        nc.scalar.activation(stg[:, 0:1024], ps[:, 0:1024], AF.Copy)
                        nc.vector.tensor_copy(stg[:, 1024:2048], ps[:, 1024:2048])
                        for b in range(BL):
                            nc.scalar.dma_start(
                                kern_raw[
                                    CH * b + 32 * h2 : CH * b + 32 * h2 + 32,
                                    128 * q + CH * b : 128 * q + CH * b + CH,
                                ],
                                stg[b : b + 1, :],
                            )
                    nc.vector.tensor_add(
                        kern[:, 128 * q : 128 * (q + 1)],
                        kern_raw[:, 128 * q : 128 * (q + 1)],
                        ab_sb[:, 128 * q : 128 * (q + 1)],
                    )

            # ---------------- phase B: conv + gate + residual ---------------
            with (
                tc.tile_pool(name="cpsum", bufs=1, space="PSUM") as cpsum,
                tc.tile_pool(name="opsum", bufs=2, space="PSUM") as opsum,
                tc.tile_pool(name="work", bufs=2) as workp,
            ):
                def emit_residual(j, zz):
                    po = opsum.tile([2 * CH, UW], F32, tag="po")
                    for h in range(UW // NT):
                        nc.tensor.matmul(
                            po[:, h * NT : (h + 1) * NT],
                            rwT_sb[:, :],
                            zz[:, h * NT : (h + 1) * NT],
                            start=True,
                            stop=True,
                        )
                    ot = workp.tile([2 * CH, UW], BF16, tag="ot")
                    nc.vector.scalar_tensor_tensor(
                        ot[:, :],
                        po[:, :],
                        rb_sb[:, 0:1],
                        xz[:, j * UW + PAD : j * UW + PAD + UW],
                        ALU.add,
                        ALU.add,
                    )
                    nc.sync.dma_start(out_d[j], ot[:, :])

                prev = None
                for j in range(NJ):
                    pt = cpsum.tile([2 * CH, UW], F32, tag="pt")
                    psg = cpsum.tile([2 * CH, UW], F32, tag="ps")
                    for g in range(2):
                        dst = pt if g == 0 else psg
                        for k in range(K):
                            q = 2 * k + g
                            for h in range(UW // NT):
                                c0 = j * UW + h * NT + DIL * k
                                nc.tensor.matmul(
                                    dst[:, h * NT : (h + 1) * NT],
                                    kern[:, 128 * q : 128 * (q + 1)],
                                    xz[:, c0 : c0 + NT],
                                    start=(k == 0),
                                    stop=(k == K - 1),
                                )
                    # residual of previous tile: its zz is ready by now, and
                    # emitting it here keeps the PE stream dense.
                    if prev is not None:
                        emit_residual(*prev)
                    ta = workp.tile([2 * CH, UW], BF16, tag="ta")
                    nc.scalar.activation(
                        ta[:, :], pt[:, :], AF.Tanh, bias=bias_t[:, 0:1]
                    )
                    ts = workp.tile([2 * CH, UW], BF16, tag="ts")
                    nc.scalar.activation(
                        ts[:, :], psg[:, :], AF.Sigmoid, bias=bias_s[:, 0:1]
                    )
                    zz = workp.tile([2 * CH, UW], BF16, tag="zz")
                    nc.vector.tensor_mul(zz[:, :], ta[:, :], ts[:, :])
                    nc.sync.dma_start(z_d[j], zz[:, :])
                    prev = (j, zz)
                emit_residual(*prev)

    nc.compile()
    return nc


def get_nc():
    global _NC
    if _NC is None:
        _NC = _build()
    return _NC


def make_in_maps(inputs):
    import ml_dtypes

    bf = ml_dtypes.bfloat16

    x = np.asarray(inputs["x"], np.float32)
    c = np.asarray(inputs["c"], np.float32)
    aw = np.asarray(inputs["adapter_w"], np.float32)
    ab = np.asarray(inputs["adapter_b"], np.float32)
    bw = np.asarray(inputs["bias_w"], np.float32)
    bb = np.asarray(inputs["bias_b"], np.float32)
    rw = np.asarray(inputs["resi_w"], np.float32)
    rb = np.asarray(inputs["resi_b"], np.float32)

    # adapter cols [cond, (g,o',i,k)] -> chunks of (i, o') per q=2k+g in QLIST order
    aw4 = aw.reshape(COND, 2, CH, CH, K)
    chunks = []
    for cidx in range(6):
        q = QLIST[cidx]
        g, k = q % 2, q // 2
        blk = aw4[:, g, :, :, k]  # [cond, o', i]
        chunks.append(np.ascontiguousarray(blk.transpose(0, 2, 1)).reshape(COND, QCH))
    aw_r = np.ascontiguousarray(np.concatenate(chunks, axis=1).astype(bf))

    # adapter bias in the block-diagonal pair layout (zeros off-diagonal)
    ab4 = ab.reshape(2, CH, CH, K)
    ab_p = np.zeros((2 * CH, 6 * 128), np.float32)
    for q in range(6):
        g, k = q % 2, q // 2
        blk = ab4[g, :, :, k].T  # [i, o']
        for b2 in range(BL):
            ab_p[CH * b2 : CH * (b2 + 1), 128 * q + CH * b2 : 128 * q + CH * b2 + CH] = blk
    ab_p = np.ascontiguousarray(ab_p.astype(bf))

    rwT_p = np.zeros((2 * CH, 2 * CH), np.float32)
    rwT_p[0:CH, 0:CH] = rw.T
    rwT_p[CH:, CH:] = rw.T
    rwT_p = np.ascontiguousarray(rwT_p.astype(bf))

    bbt = np.ascontiguousarray(np.tile(bb[0:CH], 2).reshape(2 * CH, 1))
    bbs = np.ascontiguousarray(np.tile(bb[CH:], 2).reshape(2 * CH, 1))
    rbp = np.ascontiguousarray(np.tile(rb, 2).reshape(2 * CH, 1))
    bw_b = np.ascontiguousarray(bw.astype(bf))

    in_maps = []
    for m in range(NCORES):
        sl = slice(BL * m, BL * (m + 1))
        in_maps.append(
            {
                "x_in": np.ascontiguousarray(x[sl].reshape(2 * CH, T).astype(bf)),
                "cT": np.ascontiguousarray(c[sl].T.astype(bf)),
                "aw_r": aw_r,
                "ab_p": ab_p,
                "bw": bw_b,
                "bbt": bbt,
                "bbs": bbs,
                "rwT": rwT_p,
                "rb": rbp,
            }
        )
    return in_maps


def kernel(**inputs):
    global LAST_RESULTS
    nc = get_nc()
    in_maps = make_in_maps(inputs)
    res = run_bass_kernel_spmd(nc, in_maps, list(range(NCORES)), trace=TRACE)
    LAST_RESULTS = res
    out = np.empty((B, CH, T), np.float32)
    z = np.empty((B, CH, T), np.float32)
    for m in range(NCORES):
        # [NJ, 128, UW] -> [2, CH, T]
        o = res.results[m]["out_d"].astype(np.float32)
        zt = res.results[m]["z_d"].astype(np.float32)
        out[BL * m : BL * (m + 1)] = o.transpose(1, 0, 2).reshape(BL, CH, T)
        z[BL * m : BL * (m + 1)] = zt.transpose(1, 0, 2).reshape(BL, CH, T)
    return out, z


# revision 27
# speedup vs baseline: 1.2371x; 1.1133x over previous
"""Gated TCN layer (fully conditioned) as a Bass/Tile kernel on 8 NeuronCores.

Reference computation (per sample b):
    kern = (c @ adapter_w + adapter_b).reshape(2*CH, CH, K)
    y    = dilated causal conv of x with per-sample kern (K=3, dil=4)
    y   += (c @ bias_w + bias_b)[:, None]
    z    = tanh(y[:CH]) * sigmoid(y[CH:])
    out  = resi_w @ z + resi_b + x
Returns (out, z).

Sharding: data-parallel over batch, 2 samples per core. The two samples are
packed on the 128 SBUF partitions (rows 64b+ch) so every activation / vector
op runs full-width. The per-sample conv kernels are laid out block-diagonally
per (gate, tap) so one matmul computes one gate half for both samples at once.
All matmul operands are bf16 (full PE rate + FWL); accumulation is fp32 PSUM.

DMA: transfers on one HWDGE ring execute FIFO, so the big input loads get a
dedicated ring (sync) while small scatters/constants ride the scalar/vector
rings. Output stores queue on the sync ring behind the loads.

Schedule: tanh-gate adapter chunks load first; a tanh-conv pass runs while the
sigmoid-gate chunks land; the sigmoid+residual pass is interleaved into the
tanh pass emission so the PE stream never barriers between passes.
"""

import numpy as np

from concourse import bacc, mybir, tile
from concourse.bass_utils import run_bass_kernel_spmd

K = 3
DIL = 4
CH = 64
COND = 128
B, T = 16, 16384
NCORES = 8
BL = B // NCORES          # samples per core
PAD = (K - 1) * DIL       # causal left pad = 8
NT = 512                  # matmul free-dim (one PSUM bank of fp32)
UW = 1024                 # processing unit width (2 PSUM banks)
NJ = T // UW
F = K * CH * 2 * CH       # 24576 adapter columns
QCH = CH * CH             # 4096 adapter columns per (gate, tap) block
NXC = 4                   # x load chunks
XC = T // NXC
QLIST = [0, 2, 4, 1, 3, 5]  # chunk order: tanh-gate (even q) blocks first
P1HEAD = 8                # pass-1 tiles emitted before pass-2 interleaving

F32 = mybir.dt.float32
BF16 = mybir.dt.bfloat16
AF = mybir.ActivationFunctionType
ALU = mybir.AluOpType

# Set by test.py to capture a profile; harness path leaves these alone.
TRACE = False
LAST_RESULTS = None

_NC = None


def _build():
    nc = bacc.Bacc("TRN2", target_bir_lowering=False, debug=False)

    x_in = nc.dram_tensor("x_in", [2 * CH, T], BF16, kind="ExternalInput")
    cT_d = nc.dram_tensor("cT", [COND, BL], BF16, kind="ExternalInput")
    aw_d = nc.dram_tensor("aw_r", [COND, F], BF16, kind="ExternalInput")
    ab_d = nc.dram_tensor("ab_p", [2 * CH, 6 * 128], BF16, kind="ExternalInput")
    bw_d = nc.dram_tensor("bw", [COND, 2 * CH], BF16, kind="ExternalInput")
    bbt_d = nc.dram_tensor("bbt", [2 * CH, 1], F32, kind="ExternalInput")
    bbs_d = nc.dram_tensor("bbs", [2 * CH, 1], F32, kind="ExternalInput")
    rwT_d = nc.dram_tensor("rwT", [2 * CH, 2 * CH], BF16, kind="ExternalInput")
    rb_d = nc.dram_tensor("rb", [2 * CH, 1], F32, kind="ExternalInput")
    out_d = nc.dram_tensor("out_d", [NJ, 2 * CH, UW], BF16, kind="ExternalOutput")
    z_d = nc.dram_tensor("z_d", [NJ, 2 * CH, UW], BF16, kind="ExternalOutput")

    with tile.TileContext(nc) as tc:
        with (
            tc.tile_pool(name="const", bufs=1) as constp,
            tc.tile_pool(name="xpool", bufs=1) as xpool,
            tc.tile_pool(name="kern", bufs=1) as kernp,
        ):
            # ---- sync ring: big input loads only, in priority order --------
            cT_sb = constp.tile([COND, BL], BF16)
            nc.sync.dma_start(cT_sb[:, :], cT_d[:, :])
            bw_sb = constp.tile([COND, 2 * CH], BF16)
            nc.sync.dma_start(bw_sb[:, :], bw_d[:, :])

            aw_sb = constp.tile([COND, F], BF16, name="aw_sb")
            xz = xpool.tile([2 * CH, PAD + T], BF16)
            nc.vector.memset(xz[:, 0:PAD].bitcast(F32), 0.0)

            nc.sync.dma_start(aw_sb[:, 0 : 3 * QCH], aw_d[:, 0 : 3 * QCH])
            nc.sync.dma_start(aw_sb[:, 3 * QCH : 6 * QCH], aw_d[:, 3 * QCH : 6 * QCH])
            for cx in range(NXC):
                nc.sync.dma_start(
                    xz[:, PAD + cx * XC : PAD + (cx + 1) * XC],
                    x_in[:, cx * XC : (cx + 1) * XC],
                )

            # ---- scalar ring: small constants ------------------------------
            ab_sb = constp.tile([2 * CH, 6 * 128], BF16)
            nc.scalar.dma_start(ab_sb[:, :], ab_d[:, :])
            rwT_sb = constp.tile([2 * CH, 2 * CH], BF16)
            nc.scalar.dma_start(rwT_sb[:, :], rwT_d[:, :])
            rb_sb = constp.tile([2 * CH, 1], F32)
            nc.scalar.dma_start(rb_sb[:, :], rb_d[:, :])
            bbt_sb = constp.tile([2 * CH, 1], F32)
            nc.scalar.dma_start(bbt_sb[:, :], bbt_d[:, :])
            bbs_sb = constp.tile([2 * CH, 1], F32)
            nc.scalar.dma_start(bbs_sb[:, :], bbs_d[:, :])

            # Block-diagonal per-(gate,tap) kernel tiles: block q=2k+g holds
            # lhsT[64b+i, 64b+o'] = kern[b, g*64+o', i, k].
            kern_raw = kernp.tile([2 * CH, 6 * 128], BF16, name="kern_raw")
            nc.vector.memset(kern_raw[:, :].bitcast(F32), 0.0)
            kern = kernp.tile([2 * CH, 6 * 128], BF16, name="kern")
            bias_t = kernp.tile([2 * CH, 1], F32)
            bias_s = kernp.tile([2 * CH, 1], F32)

            # ---------------- phase A: conditioned bias ---------------------
            with (
                tc.tile_pool(name="bps", bufs=1, space="PSUM") as bpsp,
                tc.tile_pool(name="bstg", bufs=1) as bstgp,
            ):
                # ~3.5us of dummy matmuls on zeroed kern_raw: trips the PE HAM
                # monitor to full clock before the adapter stream begins.
                wps = bpsp.tile([2 * CH, NT], F32, tag="warm")
                for w in range(28):
                    nc.tensor.matmul(
                        wps[:, :], kern_raw[:, 0:128], kern_raw[:, 128:128 + NT],
                        start=True, stop=True,
                    )
                pb = bpsp.tile([2 * CH, BL], F32)
                nc.tensor.matmul(pb[:, :], bw_sb[:, :], cT_sb[:, :], start=True, stop=True)
                pbs = bstgp.tile([2 * CH, BL], F32)
                nc.vector.tensor_copy(pbs[:, :], pb[:, :])
                # pair layout: rows 64b+o' = bias for sample b, out-chan o'
                nc.scalar.dma_start(bias_t[0:CH, :], pbs[0:CH, 0:1])
                nc.scalar.dma_start(bias_t[CH : 2 * CH, :], pbs[0:CH, 1:2])
                nc.scalar.dma_start(bias_s[0:CH, :], pbs[CH : 2 * CH, 0:1])
                nc.scalar.dma_start(bias_s[CH : 2 * CH, :], pbs[CH : 2 * CH, 1:2])
                nc.vector.tensor_add(bias_t[:, :], bias_t[:, :], bbt_sb[:, :])
                nc.vector.tensor_add(bias_s[:, :], bias_s[:, :], bbs_sb[:, :])

            # ---------------- phase A: adapter -> dynamic kernels -----------
            with (
                tc.tile_pool(name="apsum", bufs=2, space="PSUM") as apsum,
                tc.tile_pool(name="stg", bufs=3) as stgp,
            ):
                for cidx in range(6):
                    q = QLIST[cidx]
                    awt = aw_sb[:, cidx * QCH : (cidx + 1) * QCH]
                    for h2 in range(2):
                        ps = apsum.tile([BL, 2048], F32, tag="aps")
                        for v in range(4):
                            nc.tensor.matmul(
                                ps[:, 512 * v : 512 * (v + 1)],
                                cT_sb[:, :],
                                awt[:, 2048 * h2 + 512 * v : 2048 * h2 + 512 * (v + 1)],
                                start=True,
                                stop=True,
                            )
                        # drain PSUM with scalar and vector in parallel halves
                        stg = stgp.tile([BL, 2048], BF16, tag="stg")
                        nc.scalar.activation(stg[:, 0:1024], ps[:, 0:1024], AF.Copy)
                        nc.vector.tensor_copy(stg[:, 1024:2048], ps[:, 1024:2048])
                        for b in range(BL):
                            # split scatter triggers across two idle rings
                            (nc.scalar if b == 0 else nc.gpsimd).dma_start(
                                kern_raw[
                                    CH * b + 32 * h2 : CH * b + 32 * h2 + 32,
                                    128 * q + CH * b : 128 * q + CH * b + CH,
                                ],
                                stg[b : b + 1, :],
                            )
                    nc.vector.tensor_add(
                        kern[:, 128 * q : 128 * (q + 1)],
                        kern_raw[:, 128 * q : 128 * (q + 1)],
                        ab_sb[:, 128 * q : 128 * (q + 1)],
                    )

            # ---------------- phase B: conv + gate + residual ---------------
            with (
                tc.tile_pool(name="cpsum", bufs=1, space="PSUM") as cpsum,
                tc.tile_pool(name="opsum", bufs=2, space="PSUM") as opsum,
                tc.tile_pool(name="work", bufs=2) as workp,
            ):
                def emit_residual(j, zz):
                    po = opsum.tile([2 * CH, UW], F32, tag="po")
                    for h in range(UW // NT):
                        nc.tensor.matmul(
                            po[:, h * NT : (h + 1) * NT],
                            rwT_sb[:, :],
                            zz[:, h * NT : (h + 1) * NT],
                            start=True,
                            stop=True,
                        )
                    ot = workp.tile([2 * CH, UW], BF16, tag="ot")
                    nc.vector.scalar_tensor_tensor(
                        ot[:, :],
                        po[:, :],
                        rb_sb[:, 0:1],
                        xz[:, j * UW + PAD : j * UW + PAD + UW],
                        ALU.add,
                        ALU.add,
                    )
                    nc.sync.dma_start(out_d[j], ot[:, :])

                prev = None
                for j in range(NJ):
                    pt = cpsum.tile([2 * CH, UW], F32, tag="pt")
                    psg = cpsum.tile([2 * CH, UW], F32, tag="ps")
                    for g in range(2):
                        dst = pt if g == 0 else psg
                        for k in range(K):
                            q = 2 * k + g
                            for h in range(UW // NT):
                                c0 = j * UW + h * NT + DIL * k
                                nc.tensor.matmul(
                                    dst[:, h * NT : (h + 1) * NT],
                                    kern[:, 128 * q : 128 * (q + 1)],
                                    xz[:, c0 : c0 + NT],
                                    start=(k == 0),
                                    stop=(k == K - 1),
                                )
                    # residual of previous tile: its zz is ready by now, and
                    # emitting it here keeps the PE stream dense.
                    if prev is not None:
                        emit_residual(*prev)
                    ta = workp.tile([2 * CH, UW], BF16, tag="ta")
                    nc.scalar.activation(
                        ta[:, :], pt[:, :], AF.Tanh, bias=bias_t[:, 0:1]
                    )
                    ts = workp.tile([2 * CH, UW], BF16, tag="ts")
                    nc.scalar.activation(
                        ts[:, :], psg[:, :], AF.Sigmoid, bias=bias_s[:, 0:1]
                    )
                    zz = workp.tile([2 * CH, UW], BF16, tag="zz")
                    nc.vector.tensor_mul(zz[:, :], ta[:, :], ts[:, :])
                    nc.sync.dma_start(z_d[j], zz[:, :])
                    prev = (j, zz)
                emit_residual(*prev)

    nc.compile()
    return nc


def get_nc():
    global _NC
    if _NC is None:
        _NC = _build()
    return _NC


def make_in_maps(inputs):
    import ml_dtypes

    bf = ml_dtypes.bfloat16

    x = np.asarray(inputs["x"], np.float32)
    c = np.asarray(inputs["c"], np.float32)
    aw = np.asarray(inputs["adapter_w"], np.float32)
    ab = np.asarray(inputs["adapter_b"], np.float32)
    bw = np.asarray(inputs["bias_w"], np.float32)
    bb = np.asarray(inputs["bias_b"], np.float32)
    rw = np.asarray(inputs["resi_w"], np.float32)
    rb = np.asarray(inputs["resi_b"], np.float32)

    # adapter cols [cond, (g,o',i,k)] -> chunks of (i, o') per q=2k+g in QLIST order
    aw4 = aw.reshape(COND, 2, CH, CH, K)
    chunks = []
    for cidx in range(6):
        q = QLIST[cidx]
        g, k = q % 2, q // 2
        blk = aw4[:, g, :, :, k]  # [cond, o', i]
        chunks.append(np.ascontiguousarray(blk.transpose(0, 2, 1)).reshape(COND, QCH))
    aw_r = np.ascontiguousarray(np.concatenate(chunks, axis=1).astype(bf))

    # adapter bias in the block-diagonal pair layout (zeros off-diagonal)
    ab4 = ab.reshape(2, CH, CH, K)
    ab_p = np.zeros((2 * CH, 6 * 128), np.float32)
    for q in range(6):
        g, k = q % 2, q // 2
        blk = ab4[g, :, :, k].T  # [i, o']
        for b2 in range(BL):
            ab_p[CH * b2 : CH * (b2 + 1), 128 * q + CH * b2 : 128 * q + CH * b2 + CH] = blk
    ab_p = np.ascontiguousarray(ab_p.astype(bf))

    rwT_p = np.zeros((2 * CH, 2 * CH), np.float32)
    rwT_p[0:CH, 0:CH] = rw.T
    rwT_p[CH:, CH:] = rw.T
    rwT_p = np.ascontiguousarray(rwT_p.astype(bf))

    bbt = np.ascontiguousarray(np.tile(bb[0:CH], 2).reshape(2 * CH, 1))
    bbs = np.ascontiguousarray(np.tile(bb[CH:], 2).reshape(2 * CH, 1))
    rbp = np.ascontiguousarray(np.tile(rb, 2).reshape(2 * CH, 1))
    bw_b = np.ascontiguousarray(bw.astype(bf))

    in_maps = []
    for m in range(NCORES):
        sl = slice(BL * m, BL * (m + 1))
        in_maps.append(
            {
                "x_in": np.ascontiguousarray(x[sl].reshape(2 * CH, T).astype(bf)),
                "cT": np.ascontiguousarray(c[sl].T.astype(bf)),
                "aw_r": aw_r,
                "ab_p": ab_p,
                "bw": bw_b,
                "bbt": bbt,
                "bbs": bbs,
                "rwT": rwT_p,
                "rb": rbp,
            }
        )
    return in_maps


def kernel(**inputs):
    global LAST_RESULTS
    nc = get_nc()
    in_maps = make_in_maps(inputs)
    res = run_bass_kernel_spmd(nc, in_maps, list(range(NCORES)), trace=TRACE)
    LAST_RESULTS = res
    out = np.empty((B, CH, T), np.float32)
    z = np.empty((B, CH, T), np.float32)
    for m in range(NCORES):
        # [NJ, 128, UW] -> [2, CH, T]
        o = res.results[m]["out_d"].astype(np.float32)
        zt = res.results[m]["z_d"].astype(np.float32)
        out[BL * m : BL * (m + 1)] = o.transpose(1, 0, 2).reshape(BL, CH, T)
        z[BL * m : BL * (m + 1)] = zt.transpose(1, 0, 2).reshape(BL, CH, T)
    return out, z
